# revision 3
# baseline (speedup 1.0000x reference)
"""Invertible residual block (spectral-norm MLP + Hutchinson log-det series)
for Trainium2, data-parallel over 8 NeuronCores.

Layout strategy: feature-major on chip ([feature, batch] tiles of N=512 rows),
PE transposes at the input/output edges. The 8-term JVP series uses a composite
W0@W2' matmul to fuse consecutive terms, and accumulates sum_k coef_k * w_k
directly in PSUM via coefficient-scaled W2 stationaries; one final v-dot per
tile produces the log-det. float32r matmuls (1 cyc/row), exact-fp32 everywhere
else. Derivative-silu ACT ops are deferred and batched per 8-tile block to
amortize the ~2.7us ACT table-set switch.
"""
import sys
sys.path.insert(0, '/opt/trn_rl_repo')
from contextlib import ExitStack
import numpy as np
from concourse import bacc
import concourse.tile as tile
from concourse import mybir
from concourse.masks import make_identity

f32 = mybir.dt.float32
f32r = mybir.dt.float32r
AF = mybir.ActivationFunctionType

B, D, H = 262144, 64, 128
NCORES = 8
BC = B // NCORES          # rows per core
TN = 512                  # rows per tile
C = TN // 128             # 128-row chunks per tile
NT = BC // TN             # tiles per core
TBLK = 8                  # tiles per dsilu block
K = 8                     # series terms
N_POWER_ITERS = 5
EPS = 1e-12

_CACHE = {}


def _build_program():
    nc = bacc.Bacc("TRN2", target_bir_lowering=False)
    X = nc.dram_tensor("x", (BC, D), f32, kind="ExternalInput")
    V = nc.dram_tensor("v", (BC, D), f32, kind="ExternalInput")
    W0T = nc.dram_tensor("w0t", (D, H), f32r, kind="ExternalInput")
    W1T = nc.dram_tensor("w1t", (H, H), f32r, kind="ExternalInput")
    W2T = nc.dram_tensor("w2t", (H, D), f32r, kind="ExternalInput")
    W02T = nc.dram_tensor("w02t", (H, H), f32r, kind="ExternalInput")
    W2cT = nc.dram_tensor("w2ct", (K, H, D), f32r, kind="ExternalInput")
    B0 = nc.dram_tensor("b0", (H,), f32, kind="ExternalInput")
    B1 = nc.dram_tensor("b1", (H,), f32, kind="ExternalInput")
    B2 = nc.dram_tensor("b2", (D,), f32, kind="ExternalInput")
    Z = nc.dram_tensor("z", (BC, D), f32, kind="ExternalOutput")
    LD = nc.dram_tensor("ld", (BC,), f32, kind="ExternalOutput")

    Xr = X.rearrange("(t c p) d -> t p c d", p=128, c=C)
    Vr = V.rearrange("(t c p) d -> t p c d", p=128, c=C)
    Zr = Z.rearrange("(t c p) d -> t p c d", p=128, c=C)

    with tile.TileContext(nc) as tc:
        with ExitStack() as ctx:
            cpool = ctx.enter_context(tc.tile_pool(name="const", bufs=1))
            blk = ctx.enter_context(tc.tile_pool(name="blk", bufs=1))
            wk = ctx.enter_context(tc.tile_pool(name="wk", bufs=3))
            out2 = ctx.enter_context(tc.tile_pool(name="out2", bufs=2))
            ppt = ctx.enter_context(tc.tile_pool(name="ppt", bufs=2, space="PSUM"))
            ppb = ctx.enter_context(tc.tile_pool(name="ppb", bufs=3, space="PSUM"))
            ppg = ctx.enter_context(tc.tile_pool(name="ppg", bufs=1, space="PSUM"))
            ppS = ctx.enter_context(tc.tile_pool(name="ppS", bufs=1, space="PSUM"))
            ppl = ctx.enter_context(tc.tile_pool(name="ppl", bufs=1, space="PSUM"))

            w0t = cpool.tile([D, H], f32r)
            w1t = cpool.tile([H, H], f32r)
            w2t = cpool.tile([H, D], f32r)
            w02t = cpool.tile([H, H], f32r)
            w2ct = cpool.tile([H, K, D], f32r)
            b0 = cpool.tile([H, 1], f32)
            b1 = cpool.tile([H, 1], f32)
            b2 = cpool.tile([D, 1], f32)
            ident = cpool.tile([128, 128], f32)
            ident64 = cpool.tile([64, 64], f32)
            ones64 = cpool.tile([D, 1], f32)
            nc.sync.dma_start(out=w0t, in_=W0T[:, :])
            nc.sync.dma_start(out=w1t, in_=W1T[:, :])
            nc.sync.dma_start(out=w2t, in_=W2T[:, :])
            nc.sync.dma_start(out=w02t, in_=W02T[:, :])
            nc.sync.dma_start(out=w2ct, in_=W2cT.rearrange("k h d -> h k d"))
            nc.sync.dma_start(out=b0, in_=B0[:, None])
            nc.sync.dma_start(out=b1, in_=B1[:, None])
            nc.sync.dma_start(out=b2, in_=B2[:, None])
            make_identity(nc, ident)
            make_identity(nc, ident64)
            nc.vector.memset(ones64, 1.0)

            for blk_i in range(NT // TBLK):
                tiles = []
                # ---- phase A: load, transpose, forward, z ----
                for j in range(TBLK):
                    t = blk_i * TBLK + j
                    x_bm = blk.tile([128, C, D], f32, tag=f"xbm{j}")
                    v_bm = blk.tile([128, C, D], f32, tag=f"vbm{j}")
                    nc.sync.dma_start(out=x_bm, in_=Xr[t])
                    nc.sync.dma_start(out=v_bm, in_=Vr[t])

                    ps_tx = ppt.tile([D, TN], f32, tag="ps_t")
                    for c in range(C):
                        nc.tensor.transpose(ps_tx[:, c * 128:(c + 1) * 128],
                                            x_bm[:, c, :], ident)
                    xt = blk.tile([D, TN], f32r, tag=f"xt{j}")
                    nc.vector.tensor_copy(xt, ps_tx)

                    ps_tv = ppt.tile([D, TN], f32, tag="ps_t")
                    for c in range(C):
                        nc.tensor.transpose(ps_tv[:, c * 128:(c + 1) * 128],
                                            v_bm[:, c, :], ident)
                    vt = blk.tile([D, TN], f32r, tag=f"vt{j}")
                    nc.vector.tensor_copy(vt, ps_tv)

                    ps_a0 = ppb.tile([H, TN], f32, tag="big")
                    nc.tensor.matmul(ps_a0, w0t, xt, start=True, stop=True)
                    h0 = blk.tile([H, TN], f32r, tag=f"h0{j}")
                    nc.scalar.activation(out=h0, in_=ps_a0, func=AF.Silu, bias=b0)
                    a0c = blk.tile([H, TN], f32, tag=f"a0{j}")
                    nc.scalar.activation(out=a0c, in_=ps_a0, func=AF.Identity, bias=b0)

                    ps_a1 = ppb.tile([H, TN], f32, tag="big")
                    nc.tensor.matmul(ps_a1, w1t, h0, start=True, stop=True)
                    h1 = blk.tile([H, TN], f32r, tag=f"h1{j}")
                    nc.scalar.activation(out=h1, in_=ps_a1, func=AF.Silu, bias=b1)
                    a1c = blk.tile([H, TN], f32, tag=f"a1{j}")
                    nc.scalar.activation(out=a1c, in_=ps_a1, func=AF.Identity, bias=b1)

                    ps_g = ppg.tile([D, TN], f32, tag="gz")
                    nc.tensor.matmul(ps_g, w2t, h1, start=True, stop=True)
                    g_sb = wk.tile([D, TN], f32, tag="g")
                    nc.scalar.activation(out=g_sb, in_=ps_g, func=AF.Identity, bias=b2)

                    ps_zt = ppg.tile([128, C, D], f32, tag="gz")
                    for c in range(C):
                        nc.tensor.transpose(ps_zt[:, c, :],
                                            g_sb[:, c * 128:(c + 1) * 128], ident64)
                    z_bm = out2.tile([128, C, D], f32, tag="z")
                    nc.vector.tensor_add(z_bm, x_bm, ps_zt)
                    nc.sync.dma_start(out=Zr[t], in_=z_bm)
                    tiles.append((t, xt, vt, a0c, a1c))

                # ---- phase B: batched derivative-silu (one table switch) ----
                sds = []
                for j in range(TBLK):
                    _, _, _, a0c, a1c = tiles[j]
                    s0 = blk.tile([H, TN], f32, tag=f"s0{j}")
                    nc.scalar.activation(out=s0, in_=a0c, func=AF.Derivative_silu)
                    s1 = blk.tile([H, TN], f32, tag=f"s1{j}")
                    nc.scalar.activation(out=s1, in_=a1c, func=AF.Derivative_silu)
                    sds.append((s0, s1))

                # ---- phase C: 8-term JVP series + logdet ----
                for j in range(TBLK):
                    t, xt, vt, _, _ = tiles[j]
                    s0, s1 = sds[j]
                    t1 = None
                    ps_S = ppS.tile([D, TN], f32, tag="S")
                    for k in range(K):
                        ps_u0 = ppb.tile([H, TN], f32, tag="big")
                        if k == 0:
                            nc.tensor.matmul(ps_u0, w0t, vt, start=True, stop=True)
                        else:
                            nc.tensor.matmul(ps_u0, w02t, t1, start=True, stop=True)
                        t0 = wk.tile([H, TN], f32r, tag="t0")
                        nc.vector.tensor_mul(t0, ps_u0, s0)
                        ps_u1 = ppb.tile([H, TN], f32, tag="big")
                        nc.tensor.matmul(ps_u1, w1t, t0, start=True, stop=True)
                        u1c = wk.tile([H, TN], f32, tag="u1c")
                        nc.scalar.activation(out=u1c, in_=ps_u1, func=AF.Identity)
                        t1 = wk.tile([H, TN], f32r, tag="t1")
                        nc.gpsimd.tensor_mul(t1, u1c, s1)
                        nc.tensor.matmul(ps_S, w2ct[:, k, :], t1,
                                         start=(k == 0), stop=(k == K - 1))
                    p_sb = wk.tile([D, TN], f32, tag="p")
                    nc.vector.tensor_mul(p_sb, ps_S, vt.bitcast(f32))
                    ps_ld = ppl.tile([1, TN], f32, tag="ld")
                    nc.tensor.matmul(ps_ld, ones64, p_sb, start=True, stop=True)
                    ld_sb = out2.tile([1, TN], f32, tag="lds")
                    nc.scalar.activation(out=ld_sb, in_=ps_ld, func=AF.Identity)
                    nc.sync.dma_start(out=LD[None, t * TN:(t + 1) * TN], in_=ld_sb)

    nc.compile()
    return nc


def _spectral_normalize(W, u):
    for _ in range(N_POWER_ITERS):
        v = W.T @ u
        v = v / (np.linalg.norm(v) + EPS)
        u = W @ v
        u = u / (np.linalg.norm(u) + EPS)
    sigma = u @ (W @ v)
    return W / sigma


def _get_invoke():
    if "invoke" in _CACHE:
        return _CACHE["invoke"]
    import jax
    from jax.sharding import Mesh, PartitionSpec
    from jax.experimental.shard_map import shard_map
    from concourse import bass2jax, mybir as _mb

    nc = _build_program()
    bass2jax.install_neuronx_cc_hook()

    in_names, out_names, out_avals, zero_outs = [], [], [], []
    partition_name = nc.partition_id_tensor.name if nc.partition_id_tensor else None
    for alloc in nc.m.functions[0].allocations:
        if not isinstance(alloc, _mb.MemoryLocationSet):
            continue
        name = alloc.memorylocations[0].name
        if alloc.kind == "ExternalInput":
            if name != partition_name:
                in_names.append(name)
        elif alloc.kind == "ExternalOutput":
            out_names.append(name)
            shape = tuple(alloc.tensor_shape)
            dtype = _mb.dt.np(alloc.dtype)
            out_avals.append(jax.core.ShapedArray(shape, dtype))
            zero_outs.append(np.zeros(shape, dtype))
    n_params = len(in_names)
    all_in_names = list(in_names) + list(out_names)
    if partition_name is not None:
        all_in_names.append(partition_name)

    def _body(*args):
        operands = list(args)
        if partition_name is not None:
            operands.append(bass2jax.partition_id_tensor())
        outs = bass2jax._bass_exec_p.bind(
            *operands,
            out_avals=tuple(out_avals),
            in_names=tuple(all_in_names),
            out_names=tuple(out_names),
            lowering_input_output_aliases=(),
            sim_require_finite=True,
            sim_require_nnan=True,
            nc=nc,
        )
        return tuple(outs)

    devices = jax.devices()[:NCORES]
    mesh = Mesh(np.asarray(devices), ("core",))
    n_outs = len(out_names)
    sharded = jax.jit(
        shard_map(_body, mesh=mesh,
                  in_specs=(PartitionSpec("core"),) * (n_params + n_outs),
                  out_specs=(PartitionSpec("core"),) * n_outs,
                  check_rep=False),
        keep_unused=True,
    )
    _CACHE["invoke"] = (sharded, in_names, out_names, out_avals, zero_outs)
    return _CACHE["invoke"]


def _prep_inputs(x, hutch_v, W0, b0, W1, b1, W2, b2, u0, u1, u2):
    W0n = _spectral_normalize(np.asarray(W0, np.float32), np.asarray(u0, np.float32))
    W1n = _spectral_normalize(np.asarray(W1, np.float32), np.asarray(u1, np.float32))
    W2n = _spectral_normalize(np.asarray(W2, np.float32), np.asarray(u2, np.float32))
    W1p = (W1n / 1.1).astype(np.float32)
    W2p = (W2n / 1.1).astype(np.float32)
    coefs = [(-1.0) ** (k + 2) / (k + 1) for k in range(K)]
    per_core_const = {
        "w0t": np.ascontiguousarray(W0n.T),
        "w1t": np.ascontiguousarray(W1p.T),
        "w2t": np.ascontiguousarray(W2p.T),
        "w02t": np.ascontiguousarray((W0n @ W2p).T),
        "w2ct": np.ascontiguousarray(np.stack([c * W2p.T for c in coefs])),
        "b0": np.asarray(b0, np.float32),
        "b1": np.asarray(b1, np.float32),
        "b2": np.asarray(b2, np.float32),
    }
    xs = np.asarray(x, np.float32)
    vs = np.asarray(hutch_v, np.float32)
    return xs, vs, per_core_const


def _concat_inputs(xs, vs, const, in_names, zero_outs):
    per_core = []
    for c in range(NCORES):
        m = dict(const)
        m["x"] = xs[c * BC:(c + 1) * BC]
        m["v"] = vs[c * BC:(c + 1) * BC]
        per_core.append(m)
    concat_in = [np.concatenate([per_core[c][n] for c in range(NCORES)], axis=0)
                 for n in in_names]
    concat_zero = [np.zeros((NCORES * z.shape[0], *z.shape[1:]), z.dtype)
                   for z in zero_outs]
    return concat_in, concat_zero


def kernel(x, log_det_jacobians, hutch_v, W0, b0, W1, b1, W2, b2, u0, u1, u2):
    sharded, in_names, out_names, out_avals, zero_outs = _get_invoke()
    xs, vs, const = _prep_inputs(x, hutch_v, W0, b0, W1, b1, W2, b2, u0, u1, u2)
    concat_in, concat_zero = _concat_inputs(xs, vs, const, in_names, zero_outs)
    outs = sharded(*concat_in, *concat_zero)
    res = {name: np.asarray(outs[i]) for i, name in enumerate(out_names)}
    z = res["z"].reshape(B, D)
    ld = res["ld"].reshape(B)
    return z, np.asarray(log_det_jacobians, np.float32) + ld
